# revision 22
# baseline (speedup 1.0000x reference)
"""Cross-attention layer kernel for 8 Trainium2 NeuronCores.

Reference computation (fp32, D=1024, S=2048, B=4):
    q = x @ Wq.T + bq ; k = x @ Wk.T + bk ; v = x @ Wv.T + bv
    attn = softmax(q @ k.T / 32)
    vision = attn @ v                      # [B,S,D]
    text   = attn.T @ x                    # [B,S,D]

Sharding: core c handles batch b=c//2, query-half h=c%2 (1024 queries).
Key order inside a core is [own-half rows, other-half rows] so the
program is static; the host unpermutes when gathering.

Host-side weight algebra removes the K projection entirely:
    q k^T = xq (Wq^T Wk) x^T + rowterm(q) + colterm(k) + const
where rowterm cancels in softmax and colterm folds into the exp bias.
The host passes A = Wq^T Wk and colb = (x @ Wk^T bq) / 32.

Device dataflow (all bf16, fp32 PSUM accumulation, no DRAM spills):
  TT : T^T[d',q] = sum_d A[d,d'] xq[q,d]           (A stationary)
  S  : PT[k,q]  = exp(s * sum_d' x[k,d'] T^T[d',q] + colb[k])
       (x^T stationary, colb as per-partition activation bias)
  TR : PE-transpose PT -> P[q,k]; rowsums ride the Identity-activation
       eviction accumulator -> r = 1/rowsum
  V  : V[k,e] = x @ Wv^T + bv                      (x^T stationary)
  VIS: visionT[e,q] = sum_k V[k,e] PT[k,q]         (unnormalized; host
       divides by rowsum when gathering)
  TXT: textT[d,k] = sum_q (xq[q,d] r[q]) P[q,k]    (pair-summed on host)

All device inputs are pre-tiled on the host into partition-major
contiguous layouts (one DMA descriptor per partition) so descriptor
generation doesn't delay the first matmuls; xt/a are chunk-major so
the first chunks unblock phase TT early.
"""

import sys

import numpy as np

try:
    import concourse.bass as bass
except ImportError:  # pragma: no cover - grading env should have it on path
    sys.path.insert(0, "/opt/trn_rl_repo")
    import concourse.bass as bass

import ml_dtypes
import concourse.mybir as mybir
import concourse.tile as tile
from concourse import bacc
from concourse.bass_utils import run_bass_kernel_spmd
from concourse.masks import make_identity

F32 = mybir.dt.float32
BF16 = mybir.dt.bfloat16
FP8 = mybir.dt.float8e4
NP_BF16 = ml_dtypes.bfloat16
NP_FP8 = ml_dtypes.float8_e4m3

B = 4          # batches
S = 2048       # sequence length
D = 1024       # model dim
SH = S // 2    # queries per core
P = 128        # partitions
NT = D // P    # 8 tiles along d
NQ = SH // P   # 8 q-tiles per core
NK = S // P    # 16 k-tiles
SCALE = 1.0 / 32.0  # 1/sqrt(D)
N512 = 512


def build_program():
    nc = bacc.Bacc("TRN2", target_bir_lowering=False, debug=False, num_devices=8)

    # pre-tiled inputs: partition-major, contiguous per partition
    xt_h = nc.dram_tensor("xt", [P, 4, NT, N512], BF16, kind="ExternalInput")
    xt8_h = nc.dram_tensor("xt8", [P, 4, 2, S], FP8, kind="ExternalInput")
    xq_h = nc.dram_tensor("xq", [P, NQ, D], BF16, kind="ExternalInput")
    a_h = nc.dram_tensor("a", [P, 4, NT, 256], BF16, kind="ExternalInput")
    wvt_h = nc.dram_tensor("wvt", [P, NT, D], BF16, kind="ExternalInput")
    bv_h = nc.dram_tensor("bv", [D], F32, kind="ExternalInput")
    colb_h = nc.dram_tensor("colb", [P, NK], F32, kind="ExternalInput")

    visionT_h = nc.dram_tensor("visionT", [D, SH], BF16, kind="ExternalOutput")
    textT_h = nc.dram_tensor("textT", [D, S], BF16, kind="ExternalOutput")
    rs_h = nc.dram_tensor("rs", [SH], F32, kind="ExternalOutput")

    rs_r = rs_h.ap().rearrange("(j p) -> p j", p=P)          # [128,8]

    bv_ap = bv_h.ap()
    bv_bcast_src = bass.AP(tensor=bv_ap.tensor, offset=bv_ap.offset,
                           ap=[[0, P], bv_ap.ap[0]])         # [128,1024] bcast

    with tile.TileContext(nc) as tc:
        with (
            tc.tile_pool(name="singles", bufs=1) as singles,
            tc.tile_pool(name="dram", bufs=1, space="DRAM") as dram_pool,
            tc.tile_pool(name="xtpool", bufs=1) as xtpool,
            tc.tile_pool(name="ttpool", bufs=1) as ttpool,
            tc.tile_pool(name="ptpool", bufs=1) as ptpool,
            tc.tile_pool(name="vpool", bufs=1) as vpool,
            tc.tile_pool(name="ppool", bufs=1) as ppool,
            tc.tile_pool(name="midpool", bufs=2) as midpool,
        ):
            ident_f = singles.tile([P, P], F32)
            make_identity(nc, ident_f)
            ident = singles.tile([P, P], BF16)
            nc.vector.tensor_copy(ident, ident_f)
            colb_sb = singles.tile([P, NK], F32)
            nc.gpsimd.dma_start(out=colb_sb, in_=colb_h.ap())
            l32 = singles.tile([P, NQ * 4], F32)
            rsum = singles.tile([P, NQ], F32)
            rinv = singles.tile([P, NQ], F32)

            # resident activations
            xT = xtpool.tile([P, 4, NT, N512], BF16, name="xT")
            xt8 = xtpool.tile([P, 4, 2, S], FP8, name="xt8")
            tt8 = ttpool.tile([P, 4, 2, SH], FP8, name="tt8")
            pt = ptpool.tile([P, NK, SH], BF16, name="pt")
            v_sb = vpool.tile([P, NK, D], BF16, name="v")
            p_sb = ppool.tile([P, NQ, S], BF16, name="p")

            def xt_stat(i, td):
                """Stationary x^T slice for k-tile i, contraction tile td."""
                return xT[:, i // 4, td, (i % 4) * P:(i % 4 + 1) * P]

            # input DMA: sync (SP) and scalar (Activation) are the
            # hardware-DGE queues — put the latency-critical tiles there
            # (gpsimd descriptor generation is software, slow to start).
            a_sb = midpool.tile([P, 4, NT, 256], BF16, tag="mid", name="a")
            for c in range(4):
                nc.sync.dma_start(out=a_sb[:, c], in_=a_h.ap()[:, c])
            nc.scalar.dma_start(out=xT[:, 0], in_=xt_h.ap()[:, 0])
            nc.scalar.dma_start(out=xT[:, 1], in_=xt_h.ap()[:, 1])
            nc.sync.dma_start(out=xt8, in_=xt8_h.ap())
            nc.sync.dma_start(out=xT[:, 2], in_=xt_h.ap()[:, 2])
            nc.sync.dma_start(out=xT[:, 3], in_=xt_h.ap()[:, 3])
            wv_sb = midpool.tile([P, NT, D], BF16, tag="mid", name="wv")
            nc.scalar.dma_start(out=wv_sb, in_=wvt_h.ap())
            bvb = singles.tile([P, D], F32)
            nc.gpsimd.dma_start(out=bvb, in_=bv_bcast_src)

            # ---- phase TT: T^T = (xq @ A)^T, A stationary ---------------
            # (n outer so the n=0 half of tt is fully evicted mid-phase and
            #  phase S's first group doesn't wait on the last eviction)
            with tc.tile_pool(name="tt_ps", bufs=2, space="PSUM") as tt_ps:
                for n in range(2):
                    for tl in range(NT):
                        ps = tt_ps.tile([P, N512], F32, tag="ps")
                        for td in range(NT):
                            nc.tensor.matmul(
                                ps,
                                a_sb[:, tl // 2, td,
                                     (tl % 2) * P:(tl % 2 + 1) * P],
                                xT[:, n, td, :],
                                start=(td == 0), stop=(td == NT - 1))
                        nc.scalar.activation(
                            tt8[:, tl // 2, tl % 2,
                                n * N512:(n + 1) * N512], ps,
                            mybir.ActivationFunctionType.Identity)

            # ---- phase V: V_own = x_own @ Wv^T + bv, own k-half only ----
            # (pair-exchanged via AllGather overlapping phase S)
            v_own_d = dram_pool.tile([SH, D], BF16)
            v_gath_d = dram_pool.tile([S, D], BF16)
            with (
                tc.tile_pool(name="v_ev", bufs=4) as v_ev,
                tc.tile_pool(name="v_ps", bufs=3, space="PSUM") as v_ps,
            ):
                for i in range(NQ):
                    for h2 in range(2):
                        ps = v_ps.tile([P, N512], F32, tag="ps")
                        for td in range(NT):
                            nc.tensor.matmul(
                                ps,
                                xt_stat(i, td),
                                wv_sb[:, td, h2 * N512:(h2 + 1) * N512],
                                start=(td == 0), stop=(td == NT - 1))
                        ev = v_ev.tile([P, N512], BF16, tag="ev")
                        nc.vector.tensor_add(
                            ev, ps, bvb[:, h2 * N512:(h2 + 1) * N512])
                        nc.scalar.dma_start(
                            out=v_own_d[i * P:(i + 1) * P,
                                        h2 * N512:(h2 + 1) * N512],
                            in_=ev)
            nc.gpsimd.collective_compute(
                kind="AllGather", op=mybir.AluOpType.bypass,
                replica_groups=[[0, 1], [2, 3], [4, 5], [6, 7]],
                ins=[v_own_d], outs=[v_gath_d])
            nc.sync.dma_start(
                out=v_sb, in_=v_gath_d.rearrange("(i p) e -> p i e", p=P))

            # prefetch xq for TXT (reuses A's slot once TT is done)
            xq_sb = midpool.tile([P, NQ, D], BF16, tag="mid", name="xq")
            nc.gpsimd.dma_start(out=xq_sb, in_=xq_h.ap())

            # ---- phase S: PT = exp(s * x^T-stat @ T^T + colb) -----------
            # fp8 DoubleRow: each matmul contracts 256 (2 d-tiles packed)
            with tc.tile_pool(name="s_ps", bufs=3, space="PSUM") as s_ps:
                for i in range(NK):
                    for n in range(2):
                        ps = s_ps.tile([P, N512], F32, tag="ps")
                        for t2 in range(4):
                            nc.tensor.matmul(
                                ps,
                                xt8[:, t2, :, i * P:(i + 1) * P],
                                tt8[:, t2, :, n * N512:(n + 1) * N512],
                                start=(t2 == 0), stop=(t2 == 3),
                                perf_mode=mybir.MatmulPerfMode.DoubleRow)
                        nc.scalar.activation(
                            pt[:, i, n * N512:(n + 1) * N512], ps,
                            mybir.ActivationFunctionType.Exp,
                            bias=colb_sb[:, i:i + 1], scale=SCALE)

            # ---- phase TR: PT -> P transposes + rowsums on eviction -----
            with tc.tile_pool(name="tr_ps", bufs=2, space="PSUM") as tr_ps:
                for j in range(NQ):
                    for i4 in range(4):
                        ps = tr_ps.tile([P, 4 * P], BF16, tag="tr")
                        for c in range(4):
                            nc.tensor.transpose(
                                ps[:, c * P:(c + 1) * P],
                                pt[:, i4 * 4 + c, j * P:(j + 1) * P],
                                ident)
                        nc.scalar.activation(
                            p_sb[:, j, i4 * N512:(i4 + 1) * N512], ps,
                            mybir.ActivationFunctionType.Identity,
                            accum_out=l32[:, j * 4 + i4:j * 4 + i4 + 1])
                    nc.vector.reduce_sum(
                        out=rsum[:, j:j + 1], in_=l32[:, j * 4:(j + 1) * 4],
                        axis=mybir.AxisListType.X)
                    nc.vector.reciprocal(out=rinv[:, j:j + 1],
                                         in_=rsum[:, j:j + 1])

            nc.sync.dma_start(out=rs_r, in_=rsum)

            # xs = xq * r  (reuses Wv's slot once V is done)
            xs_sb = midpool.tile([P, NQ, D], BF16, tag="mid", name="xs")
            for j in range(NQ):
                nc.vector.tensor_scalar_mul(
                    xs_sb[:, j, :], xq_sb[:, j, :], rinv[:, j:j + 1])

            # ---- phase VIS: visionT = V-stat @ PT (unnormalized) --------
            with (
                tc.tile_pool(name="vis_ev", bufs=4) as vis_ev,
                tc.tile_pool(name="vis_ps", bufs=3, space="PSUM") as vis_ps,
            ):
                for et in range(NT):
                    for n in range(2):
                        ps = vis_ps.tile([P, N512], F32, tag="ps")
                        for i in range(NK):
                            nc.tensor.matmul(
                                ps,
                                v_sb[:, i, et * P:(et + 1) * P],
                                pt[:, i, n * N512:(n + 1) * N512],
                                start=(i == 0), stop=(i == NK - 1))
                        ev = vis_ev.tile([P, N512], BF16, tag="ev")
                        nc.vector.tensor_copy(ev, ps)
                        nc.sync.dma_start(
                            out=visionT_h.ap()[et * P:(et + 1) * P,
                                               n * N512:(n + 1) * N512],
                            in_=ev)

            # ---- phase TXT: textT = (xq*r)-stat @ P ---------------------
            with (
                tc.tile_pool(name="txt_ev", bufs=4) as txt_ev,
                tc.tile_pool(name="txt_ps", bufs=4, space="PSUM") as txt_ps,
            ):
                for dc in range(NT):
                    for kc in range(4):
                        ps = txt_ps.tile([P, N512], F32, tag="ps")
                        for j in range(NQ):
                            nc.tensor.matmul(
                                ps,
                                xs_sb[:, j, dc * P:(dc + 1) * P],
                                p_sb[:, j, kc * N512:(kc + 1) * N512],
                                start=(j == 0), stop=(j == NQ - 1))
                        ev = txt_ev.tile([P, N512], BF16, tag="ev")
                        nc.vector.tensor_copy(ev, ps)
                        nc.gpsimd.dma_start(
                            out=textT_h.ap()[dc * P:(dc + 1) * P,
                                             kc * N512:(kc + 1) * N512],
                            in_=ev)

    nc.compile()
    return nc


_NC_CACHE = []


def _get_program():
    if not _NC_CACHE:
        _NC_CACHE.append(build_program())
    return _NC_CACHE[0]


def _tile_pmajor(arr, inner):
    """[N*P, M] -> [P, N, M] partition-major tiling (N = N*P/P)."""
    n = arr.shape[0] // P
    return np.ascontiguousarray(
        arr.reshape(n, P, *arr.shape[1:]).transpose(1, 0, *range(2, arr.ndim + 1)))


def kernel(inputs, Wq, bq, Wk, bk, Wv, bv, _run_opts=None):
    x = np.asarray(inputs, dtype=np.float32)
    Wq = np.asarray(Wq, dtype=np.float32)
    bq = np.asarray(bq, dtype=np.float32)
    Wk = np.asarray(Wk, dtype=np.float32)
    bk = np.asarray(bk, dtype=np.float32)
    Wv = np.asarray(Wv, dtype=np.float32)
    bv = np.ascontiguousarray(np.asarray(bv, dtype=np.float32))

    # weight-side algebra: q k^T = xq (Wq^T Wk) x^T + rowterm + colterm
    A = (Wq.T @ Wk).astype(NP_BF16)                 # [d, d']
    # chunk-major tiling: a[p, c, t, e'] = A[t*128+p, c*256+e']
    a_t = np.ascontiguousarray(
        A.reshape(NT, P, 4, 256).transpose(1, 2, 0, 3))
    WvT = Wv.T.astype(NP_BF16)
    wv_t = np.ascontiguousarray(WvT.reshape(NT, P, D).transpose(1, 0, 2))
    w_col = Wk.T @ bq                               # [D]

    nc = _get_program()

    in_maps = []
    for c in range(8):
        b, h = divmod(c, 2)
        xb = x[b]
        perm = np.concatenate(
            [xb[h * SH:(h + 1) * SH], xb[(1 - h) * SH:(2 - h) * SH]])
        # xt8/colb/PT/P/text use NATURAL k-order so the AllGather's rank
        # order [half0, half1] matches PT tiles on both cores; the bf16 xt
        # stays own-half-first so TT/V-own slices are static.
        colb = (SCALE * (xb @ w_col)).astype(np.float32)
        colb_t = np.ascontiguousarray(colb.reshape(NK, P).T)
        xtb = perm.T.astype(NP_BF16)                # [d, s] bf16
        # xt[p, qtr, t, s'] = xtb[t*128+p, qtr*512+s']
        xt_t = np.ascontiguousarray(
            xtb.reshape(NT, P, 4, N512).transpose(1, 2, 0, 3))
        # xt8[p, t2, g, s] = fp8(x^T)[(2*t2+g)*128+p, s]  (DoubleRow pairs)
        xt8_t = np.ascontiguousarray(
            xb.T.astype(NP_FP8).reshape(4, 2, P, S).transpose(2, 0, 1, 3))
        xq_t = np.ascontiguousarray(
            perm[:SH].astype(NP_BF16).reshape(NQ, P, D).transpose(1, 0, 2))
        in_maps.append({
            "xt": xt_t, "xt8": xt8_t, "xq": xq_t, "a": a_t, "wvt": wv_t,
            "bv": bv, "colb": colb_t,
        })

    run_opts = dict(_run_opts or {})
    res = run_bass_kernel_spmd(nc, in_maps, core_ids=list(range(8)), **run_opts)
    results = res.results

    vision = np.empty((B, S, D), np.float32)
    text = np.zeros((B, S, D), np.float32)
    for c in range(8):
        b, h = divmod(c, 2)
        rs = np.asarray(results[c]["rs"], np.float32)        # [SH] rowsums
        vT = np.asarray(results[c]["visionT"], np.float32)   # [D, SH]
        vision[b, h * SH:(h + 1) * SH] = (vT / rs[None, :]).T
        tT = np.asarray(results[c]["textT"], np.float32)     # [D, S] natural
        text[b] += tT.T
    if _run_opts is not None:
        return (vision, text), res
    return (vision, text)
